# revision 8
# baseline (speedup 1.0000x reference)
# Distributed Trainium2 attention-layer kernel (8 NeuronCores).
#
# Sharding: core c in 0..7 handles (batch b = c//4, head group g = c%4).
# Each core computes q/k/v projections for its 4 heads (columns 256g:256g+256
# of Wq/Wk/Wv), rotary, scores^T, softmax (denominator via a ones-column in
# V), probs@V, and a partial out = attn_local @ Wo[rows of g]. The host sums
# the 4 group partials per batch (the tensor-parallel all-reduce, done on the
# host since full I/O passes through it anyway).
#
# Schedule: the attention inner loop is paced by ScalarE (one exp per
# (kc, head) step); all projection / output-projection matmuls are emitted as
# fine-grained fillers inside the loop so the PE never idles (keeps the HAM
# clock gate at 8/8), and the output projection for the first half of the
# sequence runs inside the last attention block. The reciprocal of the
# softmax denominator is chunked and overlapped; 1/den is broadcast across
# partitions with GpSimd so the PE does no broadcast matmuls.
#
# Self-contained: shapes hardcoded, no sibling imports.

import functools
import math

import numpy as np
import ml_dtypes

import concourse.bass as bass
import concourse.bacc as bacc
import concourse.tile as tile
import concourse.mybir as mybir
from concourse.bass_utils import run_bass_kernel_spmd

BF16 = mybir.dt.bfloat16
F32 = mybir.dt.float32

H = 16
D = 64
HID = 1024
ROT = 32
B = 2
S = 2048
NCORES = 8
HPC = 4          # heads per core
LCOL = HPC * D   # 256 local columns
NH = 512         # moving free dim per matmul

LAST_RESULT = None  # BassKernelResults of the most recent run (for test.py)


@functools.lru_cache(maxsize=4)
def _build(use_qkb: bool, use_vb: bool, use_ab: bool):
    nc = bacc.Bacc("TRN2", target_bir_lowering=False, debug=False)

    xT = nc.dram_tensor("xT", [HID, S], BF16, kind="ExternalInput")
    wq = nc.dram_tensor("wq", [HID, LCOL], BF16, kind="ExternalInput")
    wk = nc.dram_tensor("wk", [HID, LCOL], BF16, kind="ExternalInput")
    wv = nc.dram_tensor("wv", [HID, LCOL], BF16, kind="ExternalInput")
    wo = nc.dram_tensor("wo", [LCOL, HID], BF16, kind="ExternalInput")
    rotm = nc.dram_tensor("rotm", [128, S], F32, kind="ExternalInput")
    if use_qkb:
        bqd = nc.dram_tensor("bqd", [128, 2], F32, kind="ExternalInput")
        bkd = nc.dram_tensor("bkd", [128, 2], F32, kind="ExternalInput")
    if use_vb:
        bvd = nc.dram_tensor("bvd", [128, LCOL], F32, kind="ExternalInput")
    if use_ab:
        expb = nc.dram_tensor("expb", [S, S], F32, kind="ExternalInput")
    out = nc.dram_tensor("out", [S, HID], BF16, kind="ExternalOutput")

    with tile.TileContext(nc) as tc:
        with (
            tc.tile_pool(name="per", bufs=1) as per,
            tc.tile_pool(name="ex", bufs=6) as exp_pool,
            tc.tile_pool(name="asc", bufs=4) as asc_pool,
            tc.tile_pool(name="ps", bufs=2, space="PSUM") as ps,
        ):
            # ---- persistent SBUF residents ----
            xT_sb = per.tile([128, 8 * S], BF16)        # hid-chunk h at cols h*S
            wq_sb = per.tile([128, 8 * LCOL], BF16)     # hid-chunk h at cols h*LCOL
            wk_sb = per.tile([128, 8 * LCOL], BF16)
            wv_sb = per.tile([128, 8 * LCOL], BF16)
            wo_sb = per.tile([128, 2 * HID], BF16)      # col-chunk c at cols c*HID
            rotm_sb = per.tile([128, S], F32)
            qT_sb = per.tile([128, 2 * S], BF16)        # col-chunk c at cols c*S
            kT_sb = per.tile([128, 2 * S], BF16)
            v_sb = per.tile([128, 16 * (HPC * 65)], BF16)  # k-chunk kc: 4 heads x 65
            attnT_sb = per.tile([128, 2 * S], BF16)
            # head h rows parked at partition 32h; unused rows memset to 1.0
            den_sb = per.tile([97, S], F32)
            recip_sb = per.tile([97, S], F32)
            ones_sb = per.tile([97, 64], F32)

            # ---- input DMA, ordered so first projections can start early ----
            nc.sync.dma_start(wq_sb[:].rearrange("p (c n) -> p c n", c=8), wq.rearrange("(c p) n -> p c n", p=128))
            nc.sync.dma_start(wk_sb[:].rearrange("p (c n) -> p c n", c=8), wk.rearrange("(c p) n -> p c n", p=128))
            nc.sync.dma_start(rotm_sb[:], rotm[:])
            # s-half 0 of xT first (all hid chunks), then wv, then s-half 1
            for sh in range(2):
                for hch in range(8):
                    nc.sync.dma_start(
                        xT_sb[:, hch * S + sh * 1024:hch * S + (sh + 1) * 1024],
                        xT[hch * 128:(hch + 1) * 128,
                           sh * 1024:(sh + 1) * 1024],
                    )
                if sh == 0:
                    nc.sync.dma_start(wv_sb[:].rearrange("p (c n) -> p c n", c=8), wv.rearrange("(c p) n -> p c n", p=128))
            nc.sync.dma_start(wo_sb[:].rearrange("p (c n) -> p c n", c=2), wo.rearrange("(c p) n -> p c n", p=128))
            if use_qkb:
                bq_sb = per.tile([128, 2], F32)
                bk_sb = per.tile([128, 2], F32)
                nc.sync.dma_start(bq_sb[:], bqd[:])
                nc.sync.dma_start(bk_sb[:], bkd[:])
            if use_vb:
                bv_sb = per.tile([128, LCOL], F32)
                nc.sync.dma_start(bv_sb[:], bvd[:])

            nc.vector.memset(den_sb[:], 1.0)
            nc.vector.memset(ones_sb[:], 1.0)
            # preload the exp table set during the DMA head (off critical path)
            warm_ex = exp_pool.tile([1, 64], BF16, tag="warm_ex", bufs=1)
            nc.scalar.activation(warm_ex[:], ones_sb[0:1, 0:64],
                                 mybir.ActivationFunctionType.Exp, scale=0.125)
            # warm the PE clock gate during the input-DMA head: tiny
            # accumulating matmuls keep the array busy past the HAM window
            wu = ps.tile([64, 64], F32, tag="sc", name="wu")
            for i in range(40):
                nc.tensor.matmul(wu[:], ones_sb[0:1, 0:64], ones_sb[0:1, 0:64],
                                 start=(i == 0), stop=(i == 39))
            # ones columns of v (65th col of each head block)
            v_blocks = v_sb[:].rearrange("p (j c) -> p j c", c=65)
            nc.vector.memset(v_blocks[:, :, 64:65], 1.0)

            # ---------------- building blocks ----------------

            def proj_qk_half(which, c, sb2, half):
                """q/k projection sub-piece: col-chunk c, s-block sb2,
                512-wide half. ~1.7us of PE work, one PSUM bank."""
                w_sb, dst = (wq_sb, qT_sb) if which == "q" else (wk_sb, kT_sb)
                base = sb2 * 1024 + half * NH
                pp = ps.tile([128, NH], F32, tag="sc",
                             name=f"pp_{which}{c}{sb2}{half}")
                for h in range(8):
                    nc.tensor.matmul(
                        pp[:],
                        w_sb[:, h * LCOL + c * 128:h * LCOL + (c + 1) * 128],
                        xT_sb[:, h * S + base:h * S + base + NH],
                        start=(h == 0),
                        stop=(h == 7),
                    )
                if use_qkb:
                    bias_ap = (bq_sb if which == "q" else bk_sb)[:, c:c + 1]
                    nc.scalar.add(pp[:], pp[:], bias_ap)
                nc.vector.tensor_mul(
                    dst[:, c * S + base:c * S + base + NH],
                    pp[:],
                    rotm_sb[:, base:base + NH],
                )

            def proj_v(j):
                """v projection for s-chunk j (128 rows). ~0.9us PE."""
                vp = ps.tile([128, LCOL], F32, tag="sc", name=f"vp_{j}")
                for h in range(8):
                    nc.tensor.matmul(
                        vp[:],
                        xT_sb[:, h * S + j * 128:h * S + (j + 1) * 128],
                        wv_sb[:, h * LCOL:(h + 1) * LCOL],
                        start=(h == 0),
                        stop=(h == 7),
                    )
                dst = v_sb[:, j * (HPC * 65):(j + 1) * (HPC * 65)].rearrange(
                    "p (h c) -> p h c", c=65
                )[:, :, 0:64]
                src = vp[:].rearrange("p (h c) -> p h c", c=64)
                if use_vb:
                    nc.vector.tensor_add(
                        dst, src, bv_sb[:].rearrange("p (h c) -> p h c", c=64)
                    )
                else:
                    nc.vector.tensor_copy(dst, src)

            # out-projection pieces for s-block sb2 (cols sb2*1024..+1024 of
            # the query axis == output rows sb2*1024..+1024).
            rb_tiles = {}

            def recip_chunk(sb2, half):
                base = sb2 * 1024 + half * NH
                nc.vector.reciprocal(recip_sb[:, base:base + NH],
                                     den_sb[:, base:base + NH])

            def asc_make(sb2, c):
                """asc[c] = attnT * (1/den) for s-block sb2: broadcast 1/den
                across each head's 64 partitions via a K=1 f32 outer product
                (ones[1,64]^T @ recip_row), then one DVE multiply."""
                bc = ps.tile([128, 1024], F32, tag="sc", name=f"bc{sb2}{c}")
                for hi in range(2):
                    h32 = 32 * (2 * c + hi)
                    for n in range(2):
                        nc.tensor.matmul(
                            bc[hi * 64:(hi + 1) * 64, n * NH:(n + 1) * NH],
                            ones_sb[h32:h32 + 1, :],
                            recip_sb[h32:h32 + 1,
                                     sb2 * 1024 + n * NH:
                                     sb2 * 1024 + (n + 1) * NH],
                            start=True,
                            stop=True,
                            tile_position=(h32, hi * 64),
                        )
                asc = asc_pool.tile([128, 1024], BF16, tag="asc",
                                    name=f"asc{sb2}{c}")
                nc.vector.tensor_mul(
                    asc[:],
                    attnT_sb[:, c * S + sb2 * 1024:c * S + (sb2 + 1) * 1024],
                    bc[:],
                )
                return asc

            ascs_by_sb2 = {}

            def op_piece(sb2, j, use_act_store):
                """output projection for row chunk j of s-block sb2:
                [128,1024] psum, 4 matmuls, evacuate + DMA out."""
                ascs = ascs_by_sb2[sb2]
                op = ps.tile([128, 1024], F32, tag="sc", name=f"op{sb2}{j}")
                for c in range(2):
                    for n in range(2):
                        nc.tensor.matmul(
                            op[:, n * NH:(n + 1) * NH],
                            ascs[c][:, j * 128:(j + 1) * 128],
                            wo_sb[:, c * HID + n * NH:c * HID + (n + 1) * NH],
                            start=(c == 0),
                            stop=(c == 1),
                        )
                ost = asc_pool.tile([128, 1024], BF16, tag="ost", bufs=3,
                                    name=f"ost{sb2}{j}")
                if use_act_store:
                    nc.scalar.copy(ost[:], op[:])
                else:
                    nc.vector.tensor_copy(ost[:], op[:])
                nc.sync.dma_start(
                    out[sb2 * 1024 + j * 128:sb2 * 1024 + (j + 1) * 128, :],
                    ost[:],
                )

            # ---- head start: pieces needed to begin (p0, qb0) ----
            for half in range(2):
                proj_qk_half("q", 0, 0, half)
                proj_qk_half("k", 0, 0, half)
            proj_v(0)
            proj_v(1)

            # ---- fillers: emitted inside the attention loop, front-loaded.
            # Each filler is ~0.9-1.7us of independent PE work. Constraints:
            #   k(c,sb1) before kc=8 of pair c; q(c,qb) before (pair c, qb);
            #   v(j) before PV consumes k-chunk j (PV lags >=2 kc).
            fillers_by_block = {
                # (p, qb) -> list of thunks, popped one per (kc, hi) step
                (0, 0): (
                    [lambda j=j: proj_v(j) for j in range(2, 4)]
                    + [lambda h=h: proj_qk_half("k", 0, 1, h) for h in range(2)]
                    + [lambda h=h: proj_qk_half("q", 0, 1, h) for h in range(2)]
                    + [lambda j=j: proj_v(j) for j in range(4, 16)]
                    + [lambda h=h: proj_qk_half("k", 1, 0, h) for h in range(2)]
                    + [lambda h=h: proj_qk_half("k", 1, 1, h) for h in range(2)]
                ),
                (0, 1): (
                    [lambda h=h: proj_qk_half("q", 1, 0, h) for h in range(2)]
                    + [lambda h=h: proj_qk_half("q", 1, 1, h) for h in range(2)]
                ),
                (1, 0): [],
                (1, 1): [],
            }
            # v(j) ordering: v(j) must complete before PV pops k-chunk j.
            # With PV lag >= 3 steps and one filler per step starting at kc=0
            # of (p0,qb0), v(j) lands at step j-2 -> ready by PV time.

            # ---- attention: pair p = col chunk (heads 2p, 2p+1) ----
            for p in range(2):
                for qb in range(2):
                    fillers = fillers_by_block[(p, qb)]
                    outT = [
                        ps.tile([65, 1024], F32, tag="outT",
                                name=f"outT{p}{qb}{hi}")
                        for hi in range(2)
                    ]
                    pend = []  # (exp_tile, kc, hi) awaiting PV

                    def flush_pv(keep):
                        while len(pend) > keep:
                            exq, kcq, hiq = pend.pop(0)
                            hq = 2 * p + hiq
                            for n in range(2):
                                nc.tensor.matmul(
                                    outT[hiq][:, n * NH:(n + 1) * NH],
                                    v_sb[:, kcq * (HPC * 65) + hq * 65:
                                         kcq * (HPC * 65) + hq * 65 + 65],
                                    exq[:, n * NH:(n + 1) * NH],
                                    start=(kcq == 0),
                                    stop=(kcq == 15),
                                )

                    for kc in range(16):
                        for hi in range(2):
                            off = hi * 64
                            sc = ps.tile([128, 1024], F32, tag="sc",
                                         name=f"sc{p}{qb}{kc}{hi}")
                            for n in range(2):
                                nc.tensor.matmul(
                                    sc[:, n * NH:(n + 1) * NH],
                                    kT_sb[off:off + 64,
                                          p * S + kc * 128:p * S + (kc + 1) * 128],
                                    qT_sb[off:off + 64,
                                          p * S + qb * 1024 + n * NH:
                                          p * S + qb * 1024 + (n + 1) * NH],
                                    start=True,
                                    stop=True,
                                )
                            ex = exp_pool.tile([128, 1024], BF16, tag="ex",
                                               name=f"ex{p}{qb}{kc}{hi}")
                            nc.scalar.activation(
                                ex[:], sc[:], mybir.ActivationFunctionType.Exp,
                                scale=0.125,
                            )
                            if use_ab:
                                ebt = exp_pool.tile([128, 1024], F32, tag="ebt",
                                                    bufs=2, name=f"ebt{p}{qb}{kc}{hi}")
                                if hi == 0:
                                    nc.sync.dma_start(
                                        ebt[:],
                                        expb[kc * 128:(kc + 1) * 128,
                                             qb * 1024:(qb + 1) * 1024],
                                    )
                                    ebt_cur = ebt
                                nc.vector.tensor_mul(ex[:], ex[:], ebt_cur[:])
                            pend.append((ex, kc, hi))
                            # one filler per (kc, hi) step
                            if fillers:
                                fillers.pop(0)()
                            # software-pipelined PV, deep backlog for PE
                            # smoothing (ex pool bufs=6 allows lag 4)
                            flush_pv(4)
                    flush_pv(0)

                    # evacuate: den row -> den_sb, attn rows -> attnT_sb (bf16)
                    for hi in range(2):
                        h = 2 * p + hi
                        # DVE needs partition-0 dst; DMA scatters to row h.
                        dtmp = asc_pool.tile([1, 1024], F32, tag="dtmp", bufs=2,
                                             name=f"dtmp{p}{qb}{hi}")
                        nc.vector.tensor_copy(dtmp[:], outT[hi][64:65, :])
                        nc.gpsimd.dma_start(
                            den_sb[32 * h:32 * h + 1,
                                   qb * 1024:(qb + 1) * 1024], dtmp[:]
                        )
                        dst = attnT_sb[hi * 64:hi * 64 + 64,
                                       p * S + qb * 1024:p * S + (qb + 1) * 1024]
                        if hi == 0:
                            nc.vector.tensor_copy(dst, outT[hi][0:64, :])
                        else:
                            # DVE lanes can't shift partitions (0-63 ->
                            # 64-127); hop through SBUF + DMA.
                            atmp = asc_pool.tile([64, 1024], BF16, tag="atmp",
                                                 bufs=2, name=f"atmp{p}{qb}")
                            nc.vector.tensor_copy(atmp[:], outT[hi][0:64, :])
                            nc.gpsimd.dma_start(dst, atmp[:])

                    # after (p1, qb0): everything for output rows 0:1024 is
                    # known -> emit recip + broadcast, and queue the s-block-0
                    # output projection as fillers for the (p1, qb1) block.
                    if p == 1 and qb == 0:
                        recip_chunk(0, 0)
                        recip_chunk(0, 1)
                        ascs_by_sb2[0] = [asc_make(0, 0), asc_make(0, 1)]
                        f = fillers_by_block[(1, 1)]
                        for j in range(8):
                            f.append(
                                lambda j=j: op_piece(0, j, use_act_store=False))

            # ---- tail: output rows 1024:2048 ----
            recip_chunk(1, 0)
            recip_chunk(1, 1)
            ascs_by_sb2[1] = [asc_make(1, 0), asc_make(1, 1)]
            for j in range(8):
                op_piece(1, j, use_act_store=True)

    nc.compile()
    return nc


def _prep_core(c, x, sinusoids, attention_bias, Wq, bq, Wk, bk, Wv, bv, Wo,
               use_qkb, use_vb, use_ab):
    b, g = divmod(c, HPC)
    cols = slice(g * LCOL, (g + 1) * LCOL)
    bf = ml_dtypes.bfloat16
    m = {}
    m["xT"] = np.ascontiguousarray(x[b].T).astype(bf)
    m["wq"] = np.ascontiguousarray(Wq[:, cols]).astype(bf)
    m["wk"] = np.ascontiguousarray(Wk[:, cols]).astype(bf)
    m["wv"] = np.ascontiguousarray(Wv[:, cols]).astype(bf)
    m["wo"] = np.ascontiguousarray(Wo[cols, :]).astype(bf)
    sign = np.where(np.arange(ROT) % 2 == 0, -1.0, 1.0).astype(np.float32)
    mult = sinusoids[b, 1] + sign[None, :] * sinusoids[b, 0]   # [S, ROT]
    rotm = np.ones((128, S), dtype=np.float32)
    rotm[0:ROT] = mult.T
    rotm[64:64 + ROT] = mult.T
    m["rotm"] = rotm
    if use_qkb:
        m["bqd"] = np.ascontiguousarray(
            bq[cols].reshape(2, 128).T).astype(np.float32)
        m["bkd"] = np.ascontiguousarray(
            bk[cols].reshape(2, 128).T).astype(np.float32)
    if use_vb:
        m["bvd"] = np.broadcast_to(
            bv[cols].astype(np.float32), (128, LCOL)).copy()
    if use_ab:
        m["expb"] = np.ascontiguousarray(
            np.exp(attention_bias[b, 0].astype(np.float32)).T)
    return m


def kernel(x, sinusoids, attention_bias, Wq, bq, Wk, bk, Wv, bv, Wo):
    global LAST_RESULT
    x = np.asarray(x, dtype=np.float32)
    sinusoids = np.asarray(sinusoids, dtype=np.float32)
    attention_bias = np.asarray(attention_bias, dtype=np.float32)
    Wq, Wk, Wv, Wo = (np.asarray(w, dtype=np.float32) for w in (Wq, Wk, Wv, Wo))
    bq, bk, bv = (np.asarray(v, dtype=np.float32) for v in (bq, bk, bv))

    use_qkb = bool(np.any(bq) or np.any(bk))
    use_vb = bool(np.any(bv))
    use_ab = bool(np.any(attention_bias))

    nc = _build(use_qkb, use_vb, use_ab)
    in_maps = [
        _prep_core(c, x, sinusoids, attention_bias, Wq, bq, Wk, bk, Wv, bv, Wo,
                   use_qkb, use_vb, use_ab)
        for c in range(NCORES)
    ]
    import os as _os
    res = run_bass_kernel_spmd(
        nc, in_maps, core_ids=list(range(NCORES)),
        tmpdir=_os.environ.get("BASS_TMPDIR"),
    )
    LAST_RESULT = res
    outs = [r["out"].astype(np.float32) for r in res.results]
    full = np.empty((B, S, HID), dtype=np.float32)
    for b in range(B):
        full[b] = outs[4 * b] + outs[4 * b + 1] + outs[4 * b + 2] + outs[4 * b + 3]
    return full


# revision 13
# speedup vs baseline: 1.2553x; 1.2553x over previous
# Distributed Trainium2 attention-layer kernel (8 NeuronCores).
#
# Sharding: core c in 0..7 handles (batch b = c//4, head group g = c%4).
# Each core computes q/k/v projections for its 4 heads (columns 256g:256g+256
# of Wq/Wk/Wv), rotary, scores^T, softmax (denominator via a ones-column in
# V), probs@V, and a partial out = attn_local @ Wo[rows of g]. The host sums
# the 4 group partials per batch (the tensor-parallel all-reduce, done on the
# host since full I/O passes through it anyway).
#
# Schedule: the attention inner loop is paced by ScalarE (one exp per
# (kc, head) step); all projection / output-projection matmuls are emitted as
# fine-grained fillers inside the loop so the PE never idles (keeps the HAM
# clock gate at 8/8), and the output projection for the first half of the
# sequence runs inside the last attention block. The reciprocal of the
# softmax denominator is chunked and overlapped; 1/den is broadcast across
# partitions with GpSimd so the PE does no broadcast matmuls.
#
# Self-contained: shapes hardcoded, no sibling imports.

import functools
import math

import numpy as np
import ml_dtypes

import concourse.bass as bass
import concourse.bacc as bacc
import concourse.tile as tile
import concourse.mybir as mybir
from concourse.bass_utils import run_bass_kernel_spmd

BF16 = mybir.dt.bfloat16
F32 = mybir.dt.float32

H = 16
D = 64
HID = 1024
ROT = 32
B = 2
S = 2048
NCORES = 8
HPC = 4          # heads per core
LCOL = HPC * D   # 256 local columns
NH = 512         # moving free dim per matmul

LAST_RESULT = None  # BassKernelResults of the most recent run (for test.py)


@functools.lru_cache(maxsize=4)
def _build(use_qkb: bool, use_vb: bool, use_ab: bool):
    nc = bacc.Bacc("TRN2", target_bir_lowering=False, debug=False)

    xT = nc.dram_tensor("xT", [HID, S], BF16, kind="ExternalInput")
    wq = nc.dram_tensor("wq", [HID, LCOL], BF16, kind="ExternalInput")
    wk = nc.dram_tensor("wk", [HID, LCOL], BF16, kind="ExternalInput")
    wv = nc.dram_tensor("wv", [HID, LCOL], BF16, kind="ExternalInput")
    wo = nc.dram_tensor("wo", [LCOL, HID], BF16, kind="ExternalInput")
    rotm = nc.dram_tensor("rotm", [128, S], F32, kind="ExternalInput")
    if use_qkb:
        bqd = nc.dram_tensor("bqd", [128, 2], F32, kind="ExternalInput")
        bkd = nc.dram_tensor("bkd", [128, 2], F32, kind="ExternalInput")
    if use_vb:
        bvd = nc.dram_tensor("bvd", [128, LCOL], F32, kind="ExternalInput")
    if use_ab:
        expb = nc.dram_tensor("expb", [S, S], F32, kind="ExternalInput")
    out = nc.dram_tensor("out", [S, HID], BF16, kind="ExternalOutput")

    with tile.TileContext(nc) as tc:
        with (
            tc.tile_pool(name="per", bufs=1) as per,
            tc.tile_pool(name="ex", bufs=6) as exp_pool,
            tc.tile_pool(name="asc", bufs=4) as asc_pool,
            tc.tile_pool(name="ps", bufs=2, space="PSUM") as ps,
        ):
            # ---- persistent SBUF residents ----
            xT_sb = per.tile([128, 8 * S], BF16)        # hid-chunk h at cols h*S
            wq_sb = per.tile([128, 8 * LCOL], BF16)     # hid-chunk h at cols h*LCOL
            wk_sb = per.tile([128, 8 * LCOL], BF16)
            wv_sb = per.tile([128, 8 * LCOL], BF16)
            wo_sb = per.tile([128, 2 * HID], BF16)      # col-chunk c at cols c*HID
            rotm_sb = per.tile([128, S], F32)
            qT_sb = per.tile([128, 2 * S], BF16)        # col-chunk c at cols c*S
            kT_sb = per.tile([128, 2 * S], BF16)
            v_sb = per.tile([128, 16 * (HPC * 65)], BF16)  # k-chunk kc: 4 heads x 65
            attnT_sb = per.tile([128, 2 * S], BF16)
            # head h rows parked at partition 32h; unused rows memset to 1.0
            den_sb = per.tile([97, S], F32)
            recip_sb = per.tile([97, S], F32)
            ones_sb = per.tile([97, 64], F32)

            # ---- input DMA, ordered so first projections can start early:
            # wq, then xT s-half 0 (q projection can start), then wk, rotm
            # (first rotary), wv (first v proj), xT s-half 1, wo (needed last)
            nc.sync.dma_start(wq_sb[:].rearrange("p (c n) -> p c n", c=8), wq.rearrange("(c p) n -> p c n", p=128))
            for hch in range(8):
                nc.sync.dma_start(
                    xT_sb[:, hch * S:hch * S + 1024],
                    xT[hch * 128:(hch + 1) * 128, 0:1024],
                )
            nc.sync.dma_start(wk_sb[:].rearrange("p (c n) -> p c n", c=8), wk.rearrange("(c p) n -> p c n", p=128))
            nc.sync.dma_start(rotm_sb[:], rotm[:])
            nc.sync.dma_start(wv_sb[:].rearrange("p (c n) -> p c n", c=8), wv.rearrange("(c p) n -> p c n", p=128))
            for hch in range(8):
                nc.sync.dma_start(
                    xT_sb[:, hch * S + 1024:hch * S + 2048],
                    xT[hch * 128:(hch + 1) * 128, 1024:2048],
                )
            nc.sync.dma_start(wo_sb[:].rearrange("p (c n) -> p c n", c=2), wo.rearrange("(c p) n -> p c n", p=128))
            if use_qkb:
                bq_sb = per.tile([128, 2], F32)
                bk_sb = per.tile([128, 2], F32)
                nc.sync.dma_start(bq_sb[:], bqd[:])
                nc.sync.dma_start(bk_sb[:], bkd[:])
            if use_vb:
                bv_sb = per.tile([128, LCOL], F32)
                nc.sync.dma_start(bv_sb[:], bvd[:])

            nc.vector.memset(den_sb[:], 1.0)
            nc.vector.memset(ones_sb[:], 1.0)
            ones_bf = per.tile([1, 64], BF16)
            nc.vector.memset(ones_bf[:], 1.0)
            # preload the exp table set during the DMA head (off critical path)
            warm_ex = exp_pool.tile([1, 64], BF16, tag="warm_ex", bufs=1)
            nc.scalar.activation(warm_ex[:], ones_sb[0:1, 0:64],
                                 mybir.ActivationFunctionType.Exp, scale=0.125)
            # warm the PE clock gate during the input-DMA head: tiny bf16
            # accumulating matmuls (~50ns each; fp32 operands here dual-pass
            # and head-of-line block the first projections for ~10us)
            wu = ps.tile([64, 64], F32, tag="sc", name="wu")
            for i in range(40):
                nc.tensor.matmul(wu[:], ones_bf[:], ones_bf[:],
                                 start=(i == 0), stop=(i == 39))
            # ones columns of v (65th col of each head block)
            v_blocks = v_sb[:].rearrange("p (j c) -> p j c", c=65)
            nc.vector.memset(v_blocks[:, :, 64:65], 1.0)

            # ---------------- building blocks ----------------

            def proj_qk_half(which, c, sb2, half):
                """q/k projection sub-piece: col-chunk c, s-block sb2,
                512-wide half. ~1.7us of PE work, one PSUM bank."""
                w_sb, dst = (wq_sb, qT_sb) if which == "q" else (wk_sb, kT_sb)
                base = sb2 * 1024 + half * NH
                pp = ps.tile([128, NH], F32, tag="sc",
                             name=f"pp_{which}{c}{sb2}{half}")
                for h in range(8):
                    nc.tensor.matmul(
                        pp[:],
                        w_sb[:, h * LCOL + c * 128:h * LCOL + (c + 1) * 128],
                        xT_sb[:, h * S + base:h * S + base + NH],
                        start=(h == 0),
                        stop=(h == 7),
                    )
                if use_qkb:
                    bias_ap = (bq_sb if which == "q" else bk_sb)[:, c:c + 1]
                    nc.scalar.add(pp[:], pp[:], bias_ap)
                nc.vector.tensor_mul(
                    dst[:, c * S + base:c * S + base + NH],
                    pp[:],
                    rotm_sb[:, base:base + NH],
                )

            def proj_v(j):
                """v projection for s-chunk j (128 rows). ~0.9us PE."""
                vp = ps.tile([128, LCOL], F32, tag="sc", name=f"vp_{j}")
                for h in range(8):
                    nc.tensor.matmul(
                        vp[:],
                        xT_sb[:, h * S + j * 128:h * S + (j + 1) * 128],
                        wv_sb[:, h * LCOL:(h + 1) * LCOL],
                        start=(h == 0),
                        stop=(h == 7),
                    )
                dst = v_sb[:, j * (HPC * 65):(j + 1) * (HPC * 65)].rearrange(
                    "p (h c) -> p h c", c=65
                )[:, :, 0:64]
                src = vp[:].rearrange("p (h c) -> p h c", c=64)
                if use_vb:
                    nc.vector.tensor_add(
                        dst, src, bv_sb[:].rearrange("p (h c) -> p h c", c=64)
                    )
                else:
                    nc.vector.tensor_copy(dst, src)

            # out-projection pieces for s-block sb2 (cols sb2*1024..+1024 of
            # the query axis == output rows sb2*1024..+1024).
            rb_tiles = {}

            def recip_chunk(sb2, half):
                base = sb2 * 1024 + half * NH
                nc.vector.reciprocal(recip_sb[:, base:base + NH],
                                     den_sb[:, base:base + NH])

            bc_tiles = {}
            asc_tiles = {}

            def asc_half(sb2, c, half):
                """asc[c] = attnT * (1/den) for a 512-wide half of s-block
                sb2: broadcast 1/den across each head's 64 partitions via a
                K=1 f32 outer product (ones[1,64]^T @ recip_row), then one
                DVE multiply. Halves allow wave-pipelining against recip."""
                key = (sb2, c)
                if key not in bc_tiles:
                    bc_tiles[key] = ps.tile([128, 1024], F32, tag="sc",
                                            name=f"bc{sb2}{c}")
                    asc_tiles[key] = asc_pool.tile([128, 1024], BF16,
                                                   tag="asc",
                                                   name=f"asc{sb2}{c}")
                bc, asc = bc_tiles[key], asc_tiles[key]
                lo = half * NH
                for hh in range(2):
                    h32 = 32 * (2 * c + hh)
                    nc.tensor.matmul(
                        bc[hh * 64:(hh + 1) * 64, lo:lo + NH],
                        ones_sb[h32:h32 + 1, :],
                        recip_sb[h32:h32 + 1,
                                 sb2 * 1024 + lo:sb2 * 1024 + lo + NH],
                        start=True,
                        stop=True,
                        tile_position=(h32, hh * 64),
                    )
                nc.vector.tensor_mul(
                    asc[:, lo:lo + NH],
                    attnT_sb[:, c * S + sb2 * 1024 + lo:
                             c * S + sb2 * 1024 + lo + NH],
                    bc[:, lo:lo + NH],
                )

            def asc_make(sb2, c):
                asc_half(sb2, c, 0)
                asc_half(sb2, c, 1)
                return asc_tiles[(sb2, c)]

            ascs_by_sb2 = {}

            def op_piece(sb2, j, use_act_store):
                """output projection for row chunk j of s-block sb2:
                [128,1024] psum, 4 matmuls, evacuate + DMA out."""
                ascs = ascs_by_sb2[sb2]
                op = ps.tile([128, 1024], F32, tag="sc", name=f"op{sb2}{j}")
                for c in range(2):
                    for n in range(2):
                        nc.tensor.matmul(
                            op[:, n * NH:(n + 1) * NH],
                            ascs[c][:, j * 128:(j + 1) * 128],
                            wo_sb[:, c * HID + n * NH:c * HID + (n + 1) * NH],
                            start=(c == 0),
                            stop=(c == 1),
                        )
                ost = asc_pool.tile([128, 1024], BF16, tag="ost", bufs=3,
                                    name=f"ost{sb2}{j}")
                if use_act_store:
                    nc.scalar.copy(ost[:], op[:])
                else:
                    nc.vector.tensor_copy(ost[:], op[:])
                nc.sync.dma_start(
                    out[sb2 * 1024 + j * 128:sb2 * 1024 + (j + 1) * 128, :],
                    ost[:],
                )

            # ---- head start: pieces needed to begin (p0, qb0) ----
            for half in range(2):
                proj_qk_half("q", 0, 0, half)
                proj_qk_half("k", 0, 0, half)
            proj_v(0)
            proj_v(1)

            # ---- fillers: emitted inside the attention loop, front-loaded.
            # Each filler is ~0.9-1.7us of independent PE work. Constraints:
            #   k(c,sb1) before kc=8 of pair c; q(c,qb) before (pair c, qb);
            #   v(j) before PV consumes k-chunk j (PV lags >=2 kc).
            fillers_by_block = {
                # (p, qb) -> list of thunks (None = no filler this step),
                # popped one per (kc, hi) step
                (0, 0): (
                    [lambda j=j: proj_v(j) for j in range(2, 4)]
                    + [lambda h=h: proj_qk_half("k", 0, 1, h) for h in range(2)]
                    + [lambda h=h: proj_qk_half("q", 0, 1, h) for h in range(2)]
                    + [lambda j=j: proj_v(j) for j in range(4, 16)]
                ),
                (0, 1): (
                    [lambda h=h: proj_qk_half("k", 1, 0, h) for h in range(2)]
                    + [lambda h=h: proj_qk_half("k", 1, 1, h) for h in range(2)]
                    + [lambda h=h: proj_qk_half("q", 1, 0, h) for h in range(2)]
                ),
                (1, 0): [lambda h=h: proj_qk_half("q", 1, 1, h)
                         for h in range(2)],
                (1, 1): [],
            }
            # v(j) ordering: v(j) must complete before PV pops k-chunk j.
            # With PV lag >= 3 steps and one filler per step starting at kc=0
            # of (p0,qb0), v(j) lands at step j-2 -> ready by PV time.

            # ---- attention: pair p = col chunk (heads 2p, 2p+1) ----
            for p in range(2):
                for qb in range(2):
                    fillers = fillers_by_block[(p, qb)]
                    outT = [
                        ps.tile([65, 1024], F32, tag="outT",
                                name=f"outT{p}{qb}{hi}")
                        for hi in range(2)
                    ]
                    pend = []  # (exp_tile, kc, hi) awaiting PV

                    def flush_pv(keep):
                        while len(pend) > keep:
                            exq, kcq, hiq = pend.pop(0)
                            hq = 2 * p + hiq
                            for n in range(2):
                                nc.tensor.matmul(
                                    outT[hiq][:, n * NH:(n + 1) * NH],
                                    v_sb[:, kcq * (HPC * 65) + hq * 65:
                                         kcq * (HPC * 65) + hq * 65 + 65],
                                    exq[:, n * NH:(n + 1) * NH],
                                    start=(kcq == 0),
                                    stop=(kcq == 15),
                                )

                    for kc in range(16):
                        for hi in range(2):
                            off = hi * 64
                            sc = ps.tile([128, 1024], F32, tag="sc",
                                         name=f"sc{p}{qb}{kc}{hi}")
                            for n in range(2):
                                nc.tensor.matmul(
                                    sc[:, n * NH:(n + 1) * NH],
                                    kT_sb[off:off + 64,
                                          p * S + kc * 128:p * S + (kc + 1) * 128],
                                    qT_sb[off:off + 64,
                                          p * S + qb * 1024 + n * NH:
                                          p * S + qb * 1024 + (n + 1) * NH],
                                    start=True,
                                    stop=True,
                                )
                            ex = exp_pool.tile([128, 1024], BF16, tag="ex",
                                               name=f"ex{p}{qb}{kc}{hi}")
                            nc.scalar.activation(
                                ex[:], sc[:], mybir.ActivationFunctionType.Exp,
                                scale=0.125,
                            )
                            if use_ab:
                                ebt = exp_pool.tile([128, 1024], F32, tag="ebt",
                                                    bufs=2, name=f"ebt{p}{qb}{kc}{hi}")
                                if hi == 0:
                                    nc.sync.dma_start(
                                        ebt[:],
                                        expb[kc * 128:(kc + 1) * 128,
                                             qb * 1024:(qb + 1) * 1024],
                                    )
                                    ebt_cur = ebt
                                nc.vector.tensor_mul(ex[:], ex[:], ebt_cur[:])
                            pend.append((ex, kc, hi))
                            # one filler per (kc, hi) step
                            if fillers:
                                f = fillers.pop(0)
                                if f is not None:
                                    f()
                            # software-pipelined PV, deep backlog for PE
                            # smoothing (ex pool bufs=6 allows lag 4)
                            flush_pv(4)
                    flush_pv(0)

                    # evacuate. Order matters at the last two blocks: den
                    # rows first so the reciprocal (long DVE op) starts
                    # before the attnT copies queue behind it.
                    last = (p == 1 and qb == 1)
                    dtmps = []
                    for hi in range(2):
                        h = 2 * p + hi
                        # DVE needs partition-0 dst; DMA scatters to row h.
                        dtmp = asc_pool.tile([1, 1024], F32, tag="dtmp", bufs=2,
                                             name=f"dtmp{p}{qb}{hi}")
                        nc.vector.tensor_copy(dtmp[:], outT[hi][64:65, :])
                        nc.gpsimd.dma_start(
                            den_sb[32 * h:32 * h + 1,
                                   qb * 1024:(qb + 1) * 1024], dtmp[:]
                        )
                    if p == 1 and qb == 0:
                        recip_chunk(0, 0)
                        recip_chunk(0, 1)
                    if last:
                        recip_chunk(1, 0)
                    for hi in range(2):
                        dst = attnT_sb[hi * 64:hi * 64 + 64,
                                       p * S + qb * 1024:p * S + (qb + 1) * 1024]
                        if hi == 0:
                            # at the last block ScalarE is idle (post-exp);
                            # keep DVE free for the reciprocal chain.
                            if last:
                                nc.scalar.copy(dst, outT[hi][0:64, :])
                            else:
                                nc.vector.tensor_copy(dst, outT[hi][0:64, :])
                        else:
                            # DVE lanes can't shift partitions (0-63 ->
                            # 64-127); hop through SBUF + DMA.
                            atmp = asc_pool.tile([64, 1024], BF16, tag="atmp",
                                                 bufs=2, name=f"atmp{p}{qb}")
                            nc.vector.tensor_copy(atmp[:], outT[hi][0:64, :])
                            nc.gpsimd.dma_start(dst, atmp[:])

                    # after (p1, qb0): everything for output rows 0:1024 is
                    # known -> queue the s-block-0 output projection as
                    # fillers for the (p1, qb1) block (delayed a few steps so
                    # the bc broadcasts don't gate the score-slot queue on
                    # the reciprocal still running on DVE).
                    if p == 1 and qb == 0:
                        f = fillers_by_block[(1, 1)]
                        f.extend([None] * (8 - len(f)))
                        f.append(lambda: ascs_by_sb2.__setitem__(
                            0, [asc_make(0, 0), asc_make(0, 1)]))
                        for j in range(8):
                            f.append(
                                lambda j=j: op_piece(0, j, use_act_store=False))

            # ---- tail: output rows 1024:2048, wave-pipelined in 512-wide
            # halves so the output projection starts after the first
            # reciprocal chunk instead of the whole chain ----
            asc_half(1, 0, 0)
            asc_half(1, 1, 0)
            ascs_by_sb2[1] = [asc_tiles[(1, 0)], asc_tiles[(1, 1)]]
            for j in range(4):
                op_piece(1, j, use_act_store=True)
            recip_chunk(1, 1)
            asc_half(1, 0, 1)
            asc_half(1, 1, 1)
            for j in range(4, 8):
                op_piece(1, j, use_act_store=True)

    nc.compile()
    return nc


def _prep_core(c, x, sinusoids, attention_bias, Wq, bq, Wk, bk, Wv, bv, Wo,
               use_qkb, use_vb, use_ab):
    b, g = divmod(c, HPC)
    cols = slice(g * LCOL, (g + 1) * LCOL)
    bf = ml_dtypes.bfloat16
    m = {}
    m["xT"] = np.ascontiguousarray(x[b].T).astype(bf)
    m["wq"] = np.ascontiguousarray(Wq[:, cols]).astype(bf)
    m["wk"] = np.ascontiguousarray(Wk[:, cols]).astype(bf)
    m["wv"] = np.ascontiguousarray(Wv[:, cols]).astype(bf)
    m["wo"] = np.ascontiguousarray(Wo[cols, :]).astype(bf)
    sign = np.where(np.arange(ROT) % 2 == 0, -1.0, 1.0).astype(np.float32)
    mult = sinusoids[b, 1] + sign[None, :] * sinusoids[b, 0]   # [S, ROT]
    rotm = np.ones((128, S), dtype=np.float32)
    rotm[0:ROT] = mult.T
    rotm[64:64 + ROT] = mult.T
    m["rotm"] = rotm
    if use_qkb:
        m["bqd"] = np.ascontiguousarray(
            bq[cols].reshape(2, 128).T).astype(np.float32)
        m["bkd"] = np.ascontiguousarray(
            bk[cols].reshape(2, 128).T).astype(np.float32)
    if use_vb:
        m["bvd"] = np.broadcast_to(
            bv[cols].astype(np.float32), (128, LCOL)).copy()
    if use_ab:
        m["expb"] = np.ascontiguousarray(
            np.exp(attention_bias[b, 0].astype(np.float32)).T)
    return m


def kernel(x, sinusoids, attention_bias, Wq, bq, Wk, bk, Wv, bv, Wo):
    global LAST_RESULT
    x = np.asarray(x, dtype=np.float32)
    sinusoids = np.asarray(sinusoids, dtype=np.float32)
    attention_bias = np.asarray(attention_bias, dtype=np.float32)
    Wq, Wk, Wv, Wo = (np.asarray(w, dtype=np.float32) for w in (Wq, Wk, Wv, Wo))
    bq, bk, bv = (np.asarray(v, dtype=np.float32) for v in (bq, bk, bv))

    use_qkb = bool(np.any(bq) or np.any(bk))
    use_vb = bool(np.any(bv))
    use_ab = bool(np.any(attention_bias))

    nc = _build(use_qkb, use_vb, use_ab)
    in_maps = [
        _prep_core(c, x, sinusoids, attention_bias, Wq, bq, Wk, bk, Wv, bv, Wo,
                   use_qkb, use_vb, use_ab)
        for c in range(NCORES)
    ]
    import os as _os
    res = run_bass_kernel_spmd(
        nc, in_maps, core_ids=list(range(NCORES)),
        tmpdir=_os.environ.get("BASS_TMPDIR"),
    )
    LAST_RESULT = res
    outs = [r["out"].astype(np.float32) for r in res.results]
    full = np.empty((B, S, HID), dtype=np.float32)
    for b in range(B):
        full[b] = outs[4 * b] + outs[4 * b + 1] + outs[4 * b + 2] + outs[4 * b + 3]
    return full
